# revision 3
# baseline (speedup 1.0000x reference)
"""AsymDCT Trainium2 kernel.

Computes, for x (16, 64, 224, 224) f32:
  x_low  (16, 64, 112, 112) — 8x8 block DCT, keep low 4x4 coeffs, inverse with 4x4 DCT
  x_high (16, 64, 224, 224) — x minus the low-frequency reconstruction

Decomposition (per 224x224 image X, all matrices block-diagonal):
  T4 = T[:4]  (4,8);  P8 = T4^T T4;  E = t^T T4
  phase1:  M2[c,r] = sum_k X[k,c] * BT4^T[k,r]          (stationary = X chunks)
  phase2:  Q  = C1 @ BP^T   (C1 = M2^T)                 (stationary = M2 chunks)
           L2 = C1 @ BE^T
  phase3:  V  = blockdiag(T4)^T... V = B' @ Q  -> x_high = X - V
           x_low = Bt' @ L2
All PE compute in bf16 (fp32 PSUM accumulation); in/out f32.

Data parallel: 1024 images sharded 128 per NeuronCore across 8 cores.
"""

import os
import sys

import numpy as np

for _p in ("/opt/trn_rl_repo",):
    if _p not in sys.path:
        sys.path.insert(0, _p)

import ml_dtypes  # noqa: E402

import concourse.bass as bass  # noqa: E402
import concourse.mybir as mybir  # noqa: E402
from concourse import bacc  # noqa: E402
from concourse import tile  # noqa: E402
from concourse.bass_utils import run_bass_kernel_spmd  # noqa: E402

DT = mybir.dt
F32 = DT.float32
BF16 = DT.bfloat16

N_CORES = 8
BATCH, CH, IN = 16, 64, 224
LOW = 112
IMG_PER_CORE = (BATCH // N_CORES) * CH  # 128
G = 4  # images per group
N_GROUPS = IMG_PER_CORE // G  # 32

Alu = mybir.AluOpType


def dct_mat(N):
    n = np.arange(N)
    Tm = np.cos(np.pi * (2.0 * n[None, :] + 1.0) * n[:, None] / (2.0 * N))
    Tm[0, :] *= 1.0 / np.sqrt(2.0)
    Tm *= np.sqrt(2.0 / N)
    return Tm.astype(np.float32)


def build_weights(T, t):
    """Host-side block-diagonal weight matrices (fp32; cast at feed time)."""
    T = np.asarray(T, dtype=np.float64)
    t = np.asarray(t, dtype=np.float64)
    T4 = T[:4, :]
    P8 = T4.T @ T4
    E = t.T @ T4
    W4 = np.kron(np.eye(28), T4)  # (112, 224)
    return {
        "w1a": np.kron(np.eye(16), T4).T,   # (128, 64)
        "w1b": np.kron(np.eye(12), T4).T,   # (96, 48)
        "w2a": np.kron(np.eye(16), P8),     # (128, 128)
        "w2b": np.kron(np.eye(12), P8),     # (96, 96)
        "w3a": np.kron(np.eye(16), E).T,    # (128, 64)
        "w3b": np.kron(np.eye(12), E).T,    # (96, 48)
        "w4a": W4[:, :128],                 # (112, 128)
        "w4b": W4[:, 128:],                 # (112, 96)
        "w5": np.kron(np.eye(28), t),       # (112, 112)
    }


W_SHAPES = {
    "w1a": (128, 64), "w1b": (96, 48),
    "w2a": (128, 128), "w2b": (96, 96),
    "w3a": (128, 64), "w3b": (96, 48),
    "w4a": (112, 128), "w4b": (112, 96),
    "w5": (112, 112),
}


def build_nc():
    nc = bacc.Bacc("TRN2", target_bir_lowering=False, debug=False,
                   num_devices=N_CORES)

    xin = nc.declare_dram_parameter("x", [IMG_PER_CORE, IN, IN], F32,
                                    isOutput=False)
    wd = {
        k: nc.declare_dram_parameter(k, list(sh), BF16, isOutput=False)
        for k, sh in W_SHAPES.items()
    }
    xlow_d = nc.declare_dram_parameter("xlow", [IMG_PER_CORE, LOW, LOW], F32,
                                       isOutput=True)
    xhigh_d = nc.declare_dram_parameter("xhigh", [IMG_PER_CORE, IN, IN], F32,
                                        isOutput=True)

    with tile.TileContext(nc) as tc:
        with (
            tc.tile_pool(name="w", bufs=1) as wpool,
            tc.tile_pool(name="xi", bufs=3) as xpool,
            tc.tile_pool(name="m2", bufs=2) as m2pool,
            tc.tile_pool(name="qs", bufs=2) as qspool,
            tc.tile_pool(name="l2s", bufs=2) as l2pool,
            tc.tile_pool(name="xh", bufs=2) as xhpool,
            tc.tile_pool(name="xls", bufs=2) as xlspool,
            tc.tile_pool(name="pp1", bufs=1, space="PSUM") as pp1,
            tc.tile_pool(name="ppq", bufs=2, space="PSUM") as ppq,
            tc.tile_pool(name="ppl", bufs=1, space="PSUM") as ppl,
            tc.tile_pool(name="ppv", bufs=1, space="PSUM") as ppv,
            tc.tile_pool(name="ppxl", bufs=1, space="PSUM") as ppxl,
        ):
            w = {}
            for k, sh in W_SHAPES.items():
                w[k] = wpool.tile(list(sh), BF16, tag=k, name=k)
                nc.sync.dma_start(w[k][:], wd[k][:])

            for g in range(N_GROUPS):
                m = g * G

                # ---- load (cast f32 -> bf16 in DMA) ----
                xt = xpool.tile([128, G * IN], BF16, tag="xt")
                xb = xpool.tile([96, G * IN], BF16, tag="xb")
                nc.gpsimd.dma_start(
                    xt[:].rearrange("p (j c) -> p j c", j=G),
                    xin[m:m + G, 0:128, :].rearrange("j p c -> p j c"))
                nc.gpsimd.dma_start(
                    xb[:].rearrange("p (j c) -> p j c", j=G),
                    xin[m:m + G, 128:224, :].rearrange("j p c -> p j c"))

                # ---- phase 1: M2 (224, G*112) in PSUM ----
                p1t = pp1.tile([128, G * LOW], F32, tag="p1t")
                p1b = pp1.tile([96, G * LOW], F32, tag="p1b")
                for j in range(G):
                    xc = j * IN
                    mc = j * LOW
                    nc.tensor.matmul(p1t[:, mc:mc + 64],
                                     xt[:, xc:xc + 128], w["w1a"][:])
                    nc.tensor.matmul(p1b[:, mc:mc + 64],
                                     xt[:, xc + 128:xc + 224], w["w1a"][:])
                    nc.tensor.matmul(p1t[:, mc + 64:mc + 112],
                                     xb[:, xc:xc + 128], w["w1b"][:])
                    nc.tensor.matmul(p1b[:, mc + 64:mc + 112],
                                     xb[:, xc + 128:xc + 224], w["w1b"][:])

                m2t = m2pool.tile([128, G * LOW], BF16, tag="m2t")
                m2b = m2pool.tile([96, G * LOW], BF16, tag="m2b")
                nc.scalar.copy(m2t[:], p1t[:])
                nc.scalar.copy(m2b[:], p1b[:])

                # ---- phase 2: Q (112, G*224), L2 (112, G*112) ----
                l2p = ppl.tile([112, G * LOW], F32, tag="l2p")
                qs = qspool.tile([112, G * IN], BF16, tag="qs")
                xht = xhpool.tile([128, G * IN], F32, tag="xht")
                xhb = xhpool.tile([96, G * IN], F32, tag="xhb")

                for pp in range(G // 2):
                    qp = ppq.tile([112, 2 * IN], F32, tag="qp")
                    for jj in range(2):
                        j = pp * 2 + jj
                        mc = j * LOW
                        la = m2t[:, mc:mc + 112]
                        lb = m2b[:, mc:mc + 112]
                        qc = jj * IN
                        nc.tensor.matmul(qp[:, qc:qc + 128], la, w["w2a"][:])
                        nc.tensor.matmul(l2p[:, mc:mc + 64], la, w["w3a"][:])
                        nc.tensor.matmul(qp[:, qc + 128:qc + 224], lb, w["w2b"][:])
                        nc.tensor.matmul(l2p[:, mc + 64:mc + 112], lb, w["w3b"][:])
                    nc.scalar.copy(qs[:, pp * 2 * IN:(pp + 1) * 2 * IN], qp[:])

                    # ---- phase 3 high: V = B' @ Q; x_high = X - V ----
                    vt = ppv.tile([128, 2 * IN], F32, tag="vt")
                    vb = ppv.tile([96, 2 * IN], F32, tag="vb")
                    rq = qs[:, pp * 2 * IN:(pp + 1) * 2 * IN]
                    nc.tensor.matmul(vt[:], w["w4a"][:], rq)
                    nc.tensor.matmul(vb[:], w["w4b"][:], rq)
                    pc = pp * 2 * IN
                    nc.vector.scalar_tensor_tensor(
                        xht[:, pc:pc + 2 * IN], xt[:, pc:pc + 2 * IN], 1.0,
                        vt[:], Alu.mult, Alu.subtract)
                    nc.vector.scalar_tensor_tensor(
                        xhb[:, pc:pc + 2 * IN], xb[:, pc:pc + 2 * IN], 1.0,
                        vb[:], Alu.mult, Alu.subtract)

                # ---- phase 3 low: x_low = Bt' @ L2 ----
                l2s = l2pool.tile([112, G * LOW], BF16, tag="l2s")
                nc.vector.tensor_copy(l2s[:], l2p[:])
                xlp = ppxl.tile([112, G * LOW], F32, tag="xlp")
                nc.tensor.matmul(xlp[:], w["w5"][:], l2s[:])
                xls = xlspool.tile([112, G * LOW], F32, tag="xls")
                nc.scalar.copy(xls[:], xlp[:])

                # ---- stores ----
                nc.sync.dma_start(
                    xhigh_d[m:m + G, 0:128, :].rearrange("j p c -> p j c"),
                    xht[:].rearrange("p (j c) -> p j c", j=G))
                nc.sync.dma_start(
                    xhigh_d[m:m + G, 128:224, :].rearrange("j p c -> p j c"),
                    xhb[:].rearrange("p (j c) -> p j c", j=G))
                nc.sync.dma_start(
                    xlow_d[m:m + G, :, :].rearrange("j p c -> p j c"),
                    xls[:].rearrange("p (j c) -> p j c", j=G))

    nc.compile()
    return nc


_NC_CACHE = None


def _get_nc():
    global _NC_CACHE
    if _NC_CACHE is None:
        _NC_CACHE = build_nc()
    return _NC_CACHE


def run(x, T=None, t=None, trace=False):
    x = np.ascontiguousarray(np.asarray(x, dtype=np.float32))
    assert x.shape == (BATCH, CH, IN, IN), x.shape
    if T is None:
        T = dct_mat(8)
    if t is None:
        t = dct_mat(4)
    weights = build_weights(T, t)
    wmaps = {k: np.ascontiguousarray(v.astype(ml_dtypes.bfloat16))
             for k, v in weights.items()}

    per_core = BATCH // N_CORES  # 2
    in_maps = []
    for i in range(N_CORES):
        shard = x[i * per_core:(i + 1) * per_core].reshape(IMG_PER_CORE, IN, IN)
        in_maps.append({"x": np.ascontiguousarray(shard), **wmaps})

    nc = _get_nc()
    res = run_bass_kernel_spmd(nc, in_maps, core_ids=list(range(N_CORES)),
                               trace=trace)
    xlow = np.concatenate(
        [np.asarray(res.results[i]["xlow"]).reshape(per_core, CH, LOW, LOW)
         for i in range(N_CORES)], axis=0)
    xhigh = np.concatenate(
        [np.asarray(res.results[i]["xhigh"]).reshape(per_core, CH, IN, IN)
         for i in range(N_CORES)], axis=0)
    return (xlow, xhigh), res


def kernel(x, T=None, t=None):
    (xlow, xhigh), _ = run(x, T, t, trace=False)
    return (xlow, xhigh)


if __name__ == "__main__":
    nc = build_nc()
    print("built ok")
